# revision 2
# baseline (speedup 1.0000x reference)
"""CSConv2D on 8 TRN2 NeuronCores — per-pixel 5x5 kernel selection from a
25-entry bank, applied depthwise over channels, 'same' zero padding.

Sharding: data-parallel over batch B=8, one batch element per core;
per-pixel banded weights baked on the host (forward only, no collectives).

Formulation: dynamic depthwise conv as banded-stationary matmuls on the
TensorEngine with fp32 PSUM accumulation (operands bf16; rel err ~3e-3).
Geometry per core: 4 column tiles of 48 output pixels, x2 slab [104, 196*96]
packs the padded w-window twice (rows 52:104 shifted down one row), so each
output row needs NS=3 stationaries of two tap-rows each.

PSUM-drain path (the v1 bottleneck, ~90us of critical path):
  - One [112, 2048] f32 PSUM tile (4 banks) per 8-row block instead of four
    [112, 384] tiles; matmul outputs land in 512-f32-aligned slots
    (slot = (r2*NT + t)*128) so no accumulation region crosses a bank.
  - Junk partitions (48:64) are never copied: two strided copies per block
    ([0:48] and [64:112]) pull only the 96 useful columns of each slot.
    No memsets.
  - One [112, 2048] f32 PSUM tile (4 banks) per 8-row block instead of four
    [112, 384] tiles; matmul outputs land in 512-f32-aligned slots
    (slot = (r2*NT + t)*128) so no accumulation region crosses a bank.
  - Junk partitions (48:64) are never copied: two strided copies per block
    ([0:48] and [64:112]) pull only the 96 useful columns of each slot.
    No memsets.
  - Copies alternate between DVE and Act (GPSIMD cannot read PSUM).
  - DMA queues as v1: x on sync, bands and out split gpsimd/scalar.
"""

import numpy as np
import ml_dtypes

import concourse.bass as bass
import concourse.bacc as bacc
import concourse.mybir as mybir
from concourse.tile import TileContext
from concourse.bass_utils import run_bass_kernel_spmd

B, C, H, W = 8, 96, 192, 192
K, PAD = 5, 2
TW = 48
NT = W // TW          # 4 column tiles
WIN = TW + 2 * PAD    # 52 input cols per tile
K2 = 2 * WIN          # 104 packed contraction
HP = H + 2 * PAD      # 196 padded rows
NS = 3                # stationary slots per (h, t)
RB = 8                # rows per block
HB = H // RB          # 24 blocks
SLOT = 128            # psum cols per (r2, t) slot; 96 used + 32 pad
BF16 = ml_dtypes.bfloat16
N_CORES = 8

_BUILD_CACHE = {}


def build_body(nc, tc, x, bands, out):
    with (
        tc.tile_pool(name="xpool", bufs=NT) as xpool,
        tc.tile_pool(name="bpool", bufs=3) as bpool,
        tc.tile_pool(name="opool", bufs=3) as opool,
        tc.tile_pool(name="pspool", bufs=2, space="PSUM") as pspool,
    ):
        xs = []
        for t in range(NT):
            xt = xpool.tile([K2, HP * C], mybir.dt.bfloat16, tag="xslab")
            xs.append(xt)
        # x loads: row-slices issued slice-major across slabs so early
        # h-blocks' matmuls can start before the tails land.
        qr = HP // 4
        for q in range(4):
            lo = q * qr
            hi = HP if q == 3 else (q + 1) * qr
            for t in range(NT):
                nc.sync.dma_start(out=xs[t][:, lo * C : hi * C],
                                  in_=x[t][:, lo * C : hi * C])
        copy_engs = ["v", "s"]
        ci = 0
        for hb in range(HB):
            bt = bpool.tile([K2, RB * NT * NS * TW], mybir.dt.bfloat16)
            beng = nc.gpsimd if hb % 2 == 0 else nc.scalar
            beng.dma_start(out=bt, in_=bands[hb])
            ps = pspool.tile([112, (RB // 2) * NT * SLOT], mybir.dt.float32)
            for r2 in range(RB // 2):
                for par in range(2):
                    h = hb * RB + r2 * 2 + par
                    pb = par * 64
                    r = r2 * 2 + par
                    for t in range(NT):
                        co = (r2 * NT + t) * SLOT
                        for s in range(NS):
                            fo = (((r * NT) + t) * NS + s) * TW
                            nc.tensor.matmul(
                                ps[pb : pb + TW, co : co + C],
                                lhsT=bt[:, fo : fo + TW],
                                rhs=xs[t][:, (h + 2 * s) * C : (h + 2 * s + 1) * C],
                                start=(s == 0),
                                stop=(s == NS - 1),
                                skip_group_check=True,
                            )
            st = opool.tile([112, (RB // 2) * NT * C], mybir.dt.bfloat16)
            # Strided drain: [48, 16 slots, 96 used cols], skipping slot pads.
            # Engine APs need 32-aligned partition bases, so the two halves
            # stay at partitions 0:48 / 64:112 and ship as two DMAs.
            ps3 = ps.rearrange("p (n z) -> p n z", z=SLOT)
            st3 = st.rearrange("p (n c) -> p n c", c=C)
            for half in range(2):
                src = ps3[64 * half : 64 * half + 48, :, 0:C]
                dst = st3[64 * half : 64 * half + 48]
                e = copy_engs[ci % 2]
                ci += 1
                if e == "v":
                    nc.vector.tensor_copy(dst, src)
                else:
                    nc.scalar.copy(dst, src)
            oeng = nc.scalar if hb % 2 == 0 else nc.gpsimd
            oeng.dma_start(out=out[hb][0:48], in_=st[0:48])
            oeng.dma_start(out=out[hb][48:96], in_=st[64:112])


def build_bass():
    if "nc" in _BUILD_CACHE:
        return _BUILD_CACHE["nc"]
    nc = bacc.Bacc()
    x = nc.declare_dram_parameter("x", [NT, K2, HP * C], mybir.dt.bfloat16,
                                  isOutput=False)
    bands = nc.declare_dram_parameter(
        "bands", [HB, K2, RB * NT * NS * TW], mybir.dt.bfloat16, isOutput=False
    )
    out = nc.declare_dram_parameter(
        "out", [HB, 96, (RB // 2) * NT * C], mybir.dt.bfloat16, isOutput=True
    )
    with TileContext(nc) as tc:
        build_body(nc, tc, x, bands, out)
    nc.finalize()
    _BUILD_CACHE["nc"] = nc
    return nc


def prep_inputs(input, kernel_bank, buckets):
    input = np.asarray(input, dtype=np.float32)
    kernel_bank = np.asarray(kernel_bank, dtype=np.float32)
    buckets = np.asarray(buckets).astype(np.int64)

    # x2: padded transpose with one extra row so slabB = slabA shifted by +1.
    xt = input.transpose(0, 2, 3, 1)  # [B, H, W, C]
    xpad = np.zeros((B, HP + 1, W + 2 * PAD, C), np.float32)
    xpad[:, PAD : PAD + H, PAD : PAD + W, :] = xt
    xw = xpad.transpose(0, 2, 1, 3)  # [B, Wp, HP+1, C]
    cols = []
    for t in range(NT):
        slabA = xw[:, t * TW : t * TW + WIN, 0:HP]
        slabB = xw[:, t * TW : t * TW + WIN, 1 : HP + 1]
        cols.append(np.concatenate([slabA, slabB], axis=1))  # [B, 104, 196, C]
    x2 = np.stack(cols, axis=1)  # [B, NT, 104, 196, C]
    x2_bf = np.ascontiguousarray(x2.reshape(B, NT, K2, HP * C)).astype(BF16)

    # Bands: per-pixel gather + banded packing, two tap-rows per stationary.
    kbg = kernel_bank[buckets]  # [B, H, W, 5, 5]
    kbg3 = kbg.reshape(B, HB, RB, NT, TW, K, K)  # [b, hb, r, t, m, i, j]
    bnd = np.zeros((B, HB, K2, RB, NT, NS, TW), np.float32)
    marr = np.arange(TW)
    for i in range(K):
        half, slot = i % 2, i // 2
        for j in range(K):
            src = kbg3[:, :, :, :, marr, i, j]  # [B, HB, RB, NT, TW]
            bnd[:, :, half * WIN + marr + j, :, :, slot, marr] = (
                src.transpose(4, 0, 1, 2, 3)
            )
    bands_bf = bnd.reshape(B, HB, K2, RB * NT * NS * TW).astype(BF16)

    return [{"x": x2_bf[b], "bands": bands_bf[b]} for b in range(B)]


def unpack_output(outs):
    """outs: B x [HB, 96, (RB//2)*NT*C] -> [B, C, H, W] float32."""
    o = np.stack([np.asarray(a, dtype=np.float32) for a in outs]).reshape(
        B, HB, 2, TW, RB // 2, NT, C
    )
    # o[b, hb, par, m, r2, t, c] -> out[b, c, hb*RB + r2*2 + par, t*TW + m]
    out = o.transpose(0, 6, 1, 4, 2, 5, 3).reshape(B, C, H, W)
    return np.ascontiguousarray(out).astype(np.float32)


def run_spmd(in_maps, trace=False, **kwargs):
    nc = build_bass()
    return run_bass_kernel_spmd(nc, in_maps, core_ids=list(range(N_CORES)),
                                trace=trace, **kwargs)


def kernel(input, kernel_bank, buckets):
    in_maps = prep_inputs(input, kernel_bank, buckets)
    res = run_spmd(in_maps)
    return unpack_output([res.results[i]["out"] for i in range(N_CORES)])
